# revision 2
# baseline (speedup 1.0000x reference)
"""Trainium2 Bass kernel for nn_ExampleBinaryNet (binarized LeNet CNN).

Data parallel over 8 NeuronCores, 256 images each.  Numerics: conv1 input
x = fp16 hi + fp8e4m3 lo/64 (exact-sum matmuls), activations fp16.

  conv1 im2col: hi fp16 + lo fp8, 75 rows (r=(ci*5+ky)*5+kx), one
    full-tile (16-image) DMA each per batch-tile on the sync HWDGE ring.
  conv1 matmuls: per 2-image group one [128,4,512] PSUM tile (4 banks,
    8 chunks of 196 cols).  Wave 1: 8 fp16 hi matmuls; wave 2: 8 fp8 lo
    matmuls accumulating into the same chunks.  196-col chunks sustain
    ~2 col/ns (2 co-streams); a bank's first matmul carries start=True
    (has_written zeroes the whole 2KB bank), the last lo carries stop.
  epilogue: one batched ACT op per 2-image group evicts
    relu(z + b1 + 1) -> ev fp16 contiguous (1.56us/2img); 2x2 maxpool as
    half-tile DVE tensor_reduce(max, XY) + min(.,2)-1 into r2, emitted
    before conv2's DVE work so r2 completes ASAP.
  ordering: conv1(t) matmuls are emitted BEFORE conv2(t-1) so the
    in-order PE queue stays busy across the epilogue latency.
  conv2: 25 taps x 4 column groups (32-wide, zero-padded cols so all
    partition strips are written), pg shares the conv1 PSUM ring.
  fc: contiguous r2p [16,25,256], fc1 as two accumulation chains.
"""

import os
import sys

for _p in ("/opt/trn_rl_repo", "/root/.axon_site/_ro/trn_rl_repo"):
    if os.path.isdir(_p) and _p not in sys.path:
        sys.path.insert(0, _p)

import numpy as np
import ml_dtypes

import concourse.bass as bass
import concourse.tile as tile
from concourse import bacc, mybir
from concourse.bass_utils import run_bass_kernel_spmd

F32 = mybir.dt.float32
FP16 = mybir.dt.float16
FP8 = mybir.dt.float8e4
FP8NP = ml_dtypes.float8_e4m3

NCORES = 8
BPC = 256
NB = 16
NT = BPC // NB
IMW = 896
XPADT = BPC * IMW + 1024
LO_SCALE = 64.0
QW = NB * IMW          # stream elems per full tile (16 images)
NDVE = 0               # groups per tile evicted by DVE (rest on ACT)


def _build(ndve=NDVE, nolo=False, debug=False):
    nc = bacc.Bacc("TRN2", target_bir_lowering=False, debug=False)

    xh_d = nc.dram_tensor("xh", [15, XPADT], FP16, kind="ExternalInput")
    xl_d = nc.dram_tensor("xl", [15, XPADT], FP8, kind="ExternalInput")
    w1_d = nc.dram_tensor("w1t", [75, 112], FP16, kind="ExternalInput")
    w1l_d = nc.dram_tensor("w1l", [75, 112], FP8, kind="ExternalInput")
    w2_d = nc.dram_tensor("w2t", [100, 25, 32], FP16, kind="ExternalInput")
    w3_d = nc.dram_tensor("w3t", [16, 25, 120], FP16, kind="ExternalInput")
    w4_d = nc.dram_tensor("w4t", [120, 84], FP16, kind="ExternalInput")
    w5_d = nc.dram_tensor("w5t", [84, 10], FP16, kind="ExternalInput")
    b1p_d = nc.dram_tensor("b1p", [112, 1], F32, kind="ExternalInput")
    b2p_d = nc.dram_tensor("b2p", [112, 1], F32, kind="ExternalInput")
    b3p_d = nc.dram_tensor("b3p", [120, 1], F32, kind="ExternalInput")
    b4p_d = nc.dram_tensor("b4p", [84, 1], F32, kind="ExternalInput")
    b5e_d = nc.dram_tensor("b5e", [10, 1], F32, kind="ExternalInput")
    y_d = nc.dram_tensor("y", [10, BPC], F32, kind="ExternalOutput")

    NG = NB // 2  # 2-image groups per tile

    with tile.TileContext(nc) as tc:
        with (
            tc.tile_pool(name="consts", bufs=1) as consts,
            tc.tile_pool(name="hi_p", bufs=2) as hi_p,
            tc.tile_pool(name="lo_p", bufs=2) as lo_p,
            tc.tile_pool(name="ev_p", bufs=2) as ev_p,
            tc.tile_pool(name="m1_p", bufs=1) as m1_p,
            tc.tile_pool(name="r2_p", bufs=2) as r2_p,
            tc.tile_pool(name="p2_p", bufs=2) as p2_p,
            tc.tile_pool(name="fc_p", bufs=1) as fc_p,
            tc.tile_pool(name="ps1_p", bufs=2, space="PSUM") as ps1_p,
        ):
            # ---------------- constants ----------------
            w1sb = consts.tile([75, 112], FP16)
            w1lsb = consts.tile([75, 112], FP8, name="w1lsb")
            w2sb = consts.tile([100, 25, 32], FP16)
            w3sb = consts.tile([16, 25, 120], FP16)
            w4sb = consts.tile([120, 84], FP16)
            w5sb = consts.tile([84, 10], FP16)
            b1p = consts.tile([112, 1], F32)
            b2p = consts.tile([112, 1], F32)
            b3p = consts.tile([120, 1], F32)
            b4p = consts.tile([84, 1], F32)
            b5e = consts.tile([10, 1], F32)
            r2p = consts.tile([16, 25, BPC], FP16, name="r2p")

            for t_sb, t_d in [(w1sb, w1_d), (b1p, b1p_d)]:
                nc.sync.dma_start(out=t_sb, in_=t_d[:])
            nc.sync.dma_start(out=w1lsb, in_=w1l_d[:])

            def load_late_consts():
                for t_sb, t_d in [
                    (w2sb, w2_d), (w3sb, w3_d), (w4sb, w4_d), (w5sb, w5_d),
                    (b2p, b2p_d), (b3p, b3p_d), (b4p, b4p_d), (b5e, b5e_d),
                ]:
                    nc.sync.dma_start(out=t_sb, in_=t_d[:])

            def mk_src(tensor_d, offset, ap):
                return bass.AP(tensor=tensor_d.ap().tensor, offset=offset,
                               ap=ap)


            prev = None

            def conv2_block(pv):
                it, r2 = pv
                NJ = NB // 4
                pgt = ps1_p.tile([128, 4, 512], F32, name=f"pg_{it}",
                                 tag="ps1")
                pg = pgt[0:112, 0, 0:400]
                r2v = r2[:].rearrange("p (j g) (y x) -> p j g y x", g=4, x=14)
                for t in range(25):
                    ky, kx = divmod(t, 5)
                    for g in range(4):
                        rhs = r2v[:, :, g, ky : ky + 10, kx : kx + 10]
                        nc.tensor.matmul(
                            pgt[32 * g : 32 * g + 32, 0, 0:400],
                            w2sb[:, t, :],
                            rhs,
                            start=(t == 0),
                            stop=(t == 24),
                            tile_position=(0, 32 * g),
                            skip_group_check=True,
                        )
                ev2 = p2_p.tile([112, NJ, 10, 10], FP16, name=f"ev2_{it}",
                                tag="ev2")
                nc.scalar.activation(
                    out=ev2[:].rearrange("p j y x -> p (j y x)"),
                    in_=pg,
                    func=mybir.ActivationFunctionType.Relu,
                    bias=b2p[:],
                    scale=1.0,
                )
                m1c = p2_p.tile([112, NJ, 10, 5], FP16, name=f"m1c_{it}",
                                tag="m1c")
                ev2v = ev2[:].rearrange("p j y (xa xb) -> p j y xa xb", xb=2)
                nc.vector.tensor_tensor(
                    m1c[:].rearrange("p j y xa -> p (j y xa)"),
                    ev2v[:, :, :, :, 0].rearrange("p j y xa -> p (j y xa)"),
                    ev2v[:, :, :, :, 1].rearrange("p j y xa -> p (j y xa)"),
                    mybir.AluOpType.max,
                )
                m2c = p2_p.tile([112, NJ, 5, 5], FP16, name=f"m2c_{it}",
                                tag="m2c")
                m1v = m1c[:].rearrange("p j (ya yb) xa -> p j ya yb xa", yb=2)
                nc.vector.tensor_tensor(
                    m2c[:], m1v[:, :, :, 0, :], m1v[:, :, :, 1, :],
                    mybir.AluOpType.max,
                )
                # r2p[c, f, n] with n = it*16 + 4*j + g  (contiguous fc1 rhs)
                for g in range(4):
                    nc.vector.tensor_scalar(
                        out=bass.AP(
                            tensor=r2p[:].tensor,
                            offset=r2p[:].offset + it * NB + g,
                            ap=[list(r2p[:].ap[0]), [4, NJ], [BPC, 25]],
                        ),
                        in0=m2c[32 * g : 32 * g + 16].rearrange(
                            "p j a b -> p j (a b)"
                        ),
                        scalar1=2.0,
                        scalar2=1.0,
                        op0=mybir.AluOpType.min,
                        op1=mybir.AluOpType.subtract,
                    )

            for it in range(NT):
                # ---------------- im2col DMAs ----------------
                hi = hi_p.tile([75, QW], FP16, name=f"hi_{it}", tag="hi")
                lo = lo_p.tile([75, QW], FP8, name=f"lo_{it}", tag="lo")
                base = it * QW
                nq = 4 if it == 0 else 1
                for q in range(nq):
                    seg = QW // nq
                    nc.sync.dma_start(
                        out=hi[:, q * seg : (q + 1) * seg],
                        in_=mk_src(
                            xh_d, base + q * seg,
                            [[XPADT, 15], [1, 5], [1, seg]],
                        ),
                    )
                    nc.sync.dma_start(
                        out=lo[:, q * seg : (q + 1) * seg],
                        in_=mk_src(
                            xl_d, base + q * seg,
                            [[XPADT, 15], [1, 5], [1, seg]],
                        ),
                    )
                if it == 0:
                    load_late_consts()

                ev = ev_p.tile([100, NB, 28, 28], FP16, name=f"ev_{it}",
                               tag="ev")

                for g in range(NG):
                    P = ps1_p.tile([128, 4, 512], F32, name=f"P_{it}_{g}",
                                   tag="ps1")

                    def chunk_out(i, q):
                        bank = 2 * i + q // 2
                        c = q % 2
                        return P[0:112, bank, 196 * c : 196 * c + 196]

                    # wave 1: hi fp16, K=75 @ base 0
                    for i in range(2):
                        lb = 2 * g + i
                        for q in range(4):
                            rhs = bass.AP(
                                tensor=hi[:].tensor,
                                offset=hi[:].offset + lb * IMW + q * 224,
                                ap=[list(hi[:].ap[0]), [32, 7], [1, 28]],
                            )
                            nc.tensor.matmul(
                                chunk_out(i, q),
                                w1sb,
                                rhs,
                                start=(q % 2 == 0),
                                stop=(nolo and q % 2 == 1),
                                skip_group_check=True,
                            )
                    # wave 2: lo fp8 plain chunks (mirrors hi wave)
                    if not nolo:
                        for i in range(2):
                            lb = 2 * g + i
                            for q in range(4):
                                rhs = bass.AP(
                                    tensor=lo[:].tensor,
                                    offset=lo[:].offset + lb * IMW + q * 224,
                                    ap=[list(lo[:].ap[0]), [32, 7], [1, 28]],
                                )
                                nc.tensor.matmul(
                                    chunk_out(i, q),
                                    w1lsb,
                                    rhs,
                                    start=False,
                                    stop=(q % 2 == 1),
                                    skip_group_check=True,
                                )
                    # ---- evict: relu(z + b1 + 1) -> ev fp16 (batched) ----
                    p100 = P[0:100]
                    pin = bass.AP(
                        tensor=p100.tensor,
                        offset=p100.offset,
                        ap=[list(p100.ap[0]), [512, 4], [196, 2], [1, 196]],
                    )
                    evs = ev[:, 2 * g : 2 * g + 2].rearrange(
                        "p i y x -> p (i y x)"
                    )
                    if g >= NG - ndve:
                        nc.vector.tensor_scalar(
                            out=evs,
                            in0=pin,
                            scalar1=b1p[0:100],
                            scalar2=0.0,
                            op0=mybir.AluOpType.add,
                            op1=mybir.AluOpType.max,
                        )
                    else:
                        nc.scalar.activation(
                            out=evs,
                            in_=pin,
                            func=mybir.ActivationFunctionType.Relu,
                            bias=b1p[0:100],
                            scale=1.0,
                        )

                # ---------------- tile-wide pooling (DVE) ----------------
                # split into image-halves so r2 completes ASAP after the
                # last evict (conv2(t) waits on it); emitted before
                # conv2(t-1)'s DVE ops so it drains first.
                r2 = r2_p.tile([100, NB, 196], FP16, name=f"r2_{it}",
                               tag="r2")
                r2u = m1_p.tile([100, NB, 196], FP16, name=f"r2u_{it}",
                                tag="r2u")
                evx = ev[:].rearrange(
                    "p b (ya yb) (xa xb) -> p b ya xa yb xb", yb=2, xb=2
                )
                HB3 = NB // 2
                for hh in range(2):
                    sl = slice(hh * HB3, (hh + 1) * HB3)
                    nc.vector.tensor_reduce(
                        out=r2u[:, sl],
                        in_=evx[:, sl],
                        axis=mybir.AxisListType.XY,
                        op=mybir.AluOpType.max,
                    )
                    nc.vector.tensor_scalar(
                        out=r2[:, sl].rearrange("p b f -> p (b f)"),
                        in0=r2u[:, sl].rearrange("p b f -> p (b f)"),
                        scalar1=2.0,
                        scalar2=1.0,
                        op0=mybir.AluOpType.min,
                        op1=mybir.AluOpType.subtract,
                    )

                if prev is not None:
                    conv2_block(prev)

                prev = (it, r2)

            conv2_block(prev)

            # ---------------- fully connected ----------------
            HB2 = BPC // 2
            ps3 = ps1_p.tile([120, 2, 512], F32, name="ps3", tag="ps1")
            for p in range(25):
                for hh in range(2):
                    nc.tensor.matmul(
                        ps3[:, hh, 0:HB2],
                        w3sb[:, p, :],
                        r2p[:, p, hh * HB2 : (hh + 1) * HB2],
                        start=(p == 0),
                        stop=(p == 24),
                        skip_group_check=True,
                    )
            u3 = fc_p.tile([120, BPC], F32)
            nc.scalar.activation(
                out=u3[:].rearrange("p (a b) -> p a b", a=2),
                in_=ps3[:, :, 0:HB2],
                func=mybir.ActivationFunctionType.Relu,
                bias=b3p[:],
                scale=1.0,
            )
            r3 = fc_p.tile([120, BPC], FP16)
            nc.vector.tensor_scalar(
                out=r3[:], in0=u3[:], scalar1=2.0, scalar2=1.0,
                op0=mybir.AluOpType.min, op1=mybir.AluOpType.subtract,
            )

            ps4 = ps1_p.tile([84, 2, 512], F32, name="ps4", tag="ps1")
            nc.tensor.matmul(ps4[:, 0, 0:BPC], w4sb[:], r3[:],
                             start=True, stop=True, skip_group_check=True)
            u4 = fc_p.tile([84, BPC], F32)
            nc.scalar.activation(
                out=u4[:], in_=ps4[:, 0, 0:BPC],
                func=mybir.ActivationFunctionType.Relu,
                bias=b4p[:], scale=1.0,
            )
            r4 = fc_p.tile([84, BPC], FP16)
            nc.vector.tensor_scalar(
                out=r4[:], in0=u4[:], scalar1=2.0, scalar2=1.0,
                op0=mybir.AluOpType.min, op1=mybir.AluOpType.subtract,
            )

            ps5 = ps1_p.tile([10, 2, 512], F32, name="ps5", tag="ps1")
            nc.tensor.matmul(ps5[:, 0, 0:BPC], w5sb[:], r4[:],
                             start=True, stop=True, skip_group_check=True)
            y_sb = fc_p.tile([10, BPC], F32)
            nc.vector.tensor_scalar_add(y_sb[:], ps5[:, 0, 0:BPC], b5e[:])
            nc.sync.dma_start(out=y_d[:], in_=y_sb[:])

    nc.compile()
    return nc


_NC_CACHE = {}


def _get_nc(ndve=NDVE, nolo=False, debug=False):
    key = (ndve, nolo, debug)
    if key not in _NC_CACHE:
        _NC_CACHE[key] = _build(ndve, nolo, debug)
    return _NC_CACHE[key]


def _prep_weights(w1, b1, w2, b2, w3, b3, w4, b4, w5, b5):
    s1 = np.sign(w1).astype(np.float32)  # [100,3,5,5]
    s2 = np.sign(w2).astype(np.float32)
    s3 = np.sign(w3).astype(np.float32)
    s4 = np.sign(w4).astype(np.float32)
    s5 = np.sign(w5).astype(np.float32)

    # hi lhsT rows: r = (ci*5+ky)*5 + kx
    w1t = np.zeros((75, 112), np.float32)
    w1t[:, :100] = s1.transpose(1, 2, 3, 0).reshape(75, 100)
    w1l = w1t / LO_SCALE
    w2t = np.zeros((100, 25, 32), np.float16)
    w2t[:, :, :16] = s2.transpose(1, 2, 3, 0).reshape(100, 25, 16)
    w3t = np.ascontiguousarray(
        s3.reshape(120, 16, 25).transpose(1, 2, 0)
    ).astype(np.float16)
    w4t = np.ascontiguousarray(s4.T).astype(np.float16)
    w5t = np.ascontiguousarray(s5.T).astype(np.float16)

    def colvec(v, n):
        out = np.zeros((n, 1), np.float32)
        out[: len(v), 0] = v
        return out

    b1p = colvec(b1 + 1.0, 112)
    b2p = np.zeros((112, 1), np.float32)
    for g in range(4):
        b2p[32 * g : 32 * g + 16, 0] = b2 + 1.0
    b3p = colvec(b3 + 1.0, 120)
    b4p = colvec(b4 + 1.0, 84)
    b5e = colvec(b5, 10)
    return {
        "w1t": w1t.astype(np.float16),
        "w1l": w1l.astype(FP8NP),
        "w2t": w2t, "w3t": w3t, "w4t": w4t, "w5t": w5t,
        "b1p": b1p, "b2p": b2p, "b3p": b3p, "b4p": b4p, "b5e": b5e,
    }


def kernel(x, w1, b1, w2, b2, w3, b3, w4, b4, w5, b5, _trace=False,
           _ndve=NDVE, _nolo=False, _debug=False):
    x = np.asarray(x, dtype=np.float32)
    wmap = _prep_weights(
        np.asarray(w1), np.asarray(b1), np.asarray(w2), np.asarray(b2),
        np.asarray(w3), np.asarray(b3), np.asarray(w4), np.asarray(b4),
        np.asarray(w5), np.asarray(b5),
    )
    nc = _get_nc(_ndve, _nolo, _debug)
    in_maps = []
    for c in range(NCORES):
        xs = x[c * BPC : (c + 1) * BPC]  # [256,3,32,32]
        xs = np.ascontiguousarray(
            xs.transpose(1, 0, 2, 3).reshape(3, BPC * 1024)
        )
        xh0 = np.zeros((3, BPC * 1024 + 1024), np.float16)
        xh0[:, : BPC * 1024] = xs.astype(np.float16)
        xl0 = np.zeros((3, BPC * 1024 + 1024), FP8NP)
        xl0[:, : BPC * 1024] = (
            (xs - xh0[:, : BPC * 1024].astype(np.float32)) * LO_SCALE
        ).astype(FP8NP)
        xh = np.zeros((15, XPADT), np.float16)
        xl = np.zeros((15, XPADT), FP8NP)
        for ci in range(3):
            for ky in range(5):
                s = 32 * ky
                a = ci * 5 + ky
                sh = xh0[ci, s : s + BPC * 1024].reshape(BPC, 1024)
                sl = xl0[ci, s : s + BPC * 1024].reshape(BPC, 1024)
                xh[a, : BPC * IMW] = sh[:, :IMW].ravel()
                xl[a, : BPC * IMW] = sl[:, :IMW].ravel()
        in_maps.append({"xh": xh, "xl": xl, **wmap})
    res = run_bass_kernel_spmd(
        nc, in_maps, list(range(NCORES)), trace=_trace
    )
    out = np.empty((NCORES * BPC, 10), np.float32)
    for c in range(NCORES):
        out[c * BPC : (c + 1) * BPC] = res.results[c]["y"].T
    if _trace:
        return out, res
    return out
